# revision 2
# baseline (speedup 1.0000x reference)
"""MultiHeadLatentAttention prefill kernel for 8 Trainium2 NeuronCores.

Sharding v2: head-group tensor parallelism. Core j handles batch j//4 and
head group j%4 (4 of 16 heads), computing attention for its heads over the
FULL sequence with causal strip-skipping (query block qb only visits key
strips <= diagonal, so scores/attn@v/exp do the minimal lower-triangle work).
Out-projection is row-parallel over heads: each core emits a full [T, D]
fp32 partial product; the host sums the 4 partials per batch at unshard time
(the all-reduce implied by row-parallel W_out, done on the host since the
SPMD harness has no cross-core collectives).

x^T is produced by the DMA xbar transpose engine (dma_start_transpose)
directly from DRAM, freeing the PE/DVE from 256 transpose+copy pairs.
All matmuls run bf16 with fp32 PSUM accumulation. Softmax skips
max-subtraction (|scores| <= ~1.3 here) and gets denominators from a ones
column appended to each head's V block. The emission schedule interleaves
projection / out-proj matmuls ("filler units") into the attention inner
loops so the PE never waits on the ACT engine's exp stream.
"""
import sys

sys.path.insert(0, "/opt/trn_rl_repo")

import numpy as np
import ml_dtypes

import concourse.bass as bass
import concourse.bacc as bacc
import concourse.mybir as mybir
import concourse.tile as tile
from concourse import bass_utils
from concourse.masks import make_identity

BF16 = ml_dtypes.bfloat16

B, T, D = 2, 2048, 2048
H, HD, L = 16, 128, 256
G = 4                      # heads per core
GW = G * HD                # 512: width of this core's head-group slice
NQB = 4                    # query blocks of 512
NSTRIP = 16                # key strips of 128
N_CORES = 8
SCALE = 1.0 / np.sqrt(HD)

DT = mybir.dt.bfloat16
F32 = mybir.dt.float32


def _build_module():
    nc = bacc.Bacc("TRN2", target_bir_lowering=False, debug=False)

    xk_d = nc.dram_tensor("xk", [T, D], DT, kind="ExternalInput")
    wq_d = nc.dram_tensor("wq", [D, GW], DT, kind="ExternalInput")
    wd_d = nc.dram_tensor("wd", [D, L], DT, kind="ExternalInput")
    wuk_d = nc.dram_tensor("wuk", [L, GW], DT, kind="ExternalInput")
    wuv_d = nc.dram_tensor("wuv", [L, GW], DT, kind="ExternalInput")
    wo_d = nc.dram_tensor("wo", [GW, D], DT, kind="ExternalInput")
    # triangular mask for the 4 diagonal strips of a 512-query block
    mask_d = nc.dram_tensor("mask", [4, 128, 512], DT, kind="ExternalInput")
    out_d = nc.dram_tensor("out", [T, D], F32, kind="ExternalOutput")

    with tile.TileContext(nc) as tc:
        with (
            tc.tile_pool(name="sb", bufs=1) as psb,
            tc.tile_pool(name="ps", bufs=3, space="PSUM") as pps,
            tc.tile_pool(name="ctx", bufs=1, space="PSUM") as pctx,
        ):
            ident = psb.tile([128, 128], DT)
            make_identity(nc, ident[:])

            # ---- DMA queue (order = need order) --------------------------
            wd_sb = psb.tile([128, 16 * L], DT)
            nc.sync.dma_start(
                wd_sb[:].rearrange("p (t c) -> p t c", c=L),
                wd_d.ap().rearrange("(t p) c -> p t c", p=128),
            )
            mask_sb = psb.tile([128, 4 * 512], DT)
            for j in range(4):
                nc.sync.dma_start(mask_sb[:, j * 512 : (j + 1) * 512], mask_d.ap()[j])
            wuk_sb = psb.tile([128, 2 * GW], DT)  # [lat-in-tile, ltile*GW]
            nc.sync.dma_start(
                wuk_sb[:].rearrange("p (t c) -> p t c", c=GW),
                wuk_d.ap().rearrange("(t p) c -> p t c", p=128),
            )
            wuv_sb = psb.tile([128, 2 * GW], DT)
            nc.sync.dma_start(
                wuv_sb[:].rearrange("p (t c) -> p t c", c=GW),
                wuv_d.ap().rearrange("(t p) c -> p t c", p=128),
            )

            xkT = psb.tile([128, 16 * T], DT)  # [d-in-tile, dtile*T]

            def xbar(c):
                # xkT[p, dd, q] = x[q, dd*128+p] for q in chunk c
                nc.sync.dma_start_transpose(
                    xkT[:].rearrange("p (t q) -> p t q", q=T)[
                        :, :, c * 512 : (c + 1) * 512
                    ],
                    xk_d.ap()[c * 512 : (c + 1) * 512, :],
                )

            xbar(0)
            wq_tiles = []
            for h in range(G):
                wq_h = psb.tile([128, 16 * 128], DT, tag="wq", bufs=G, name=f"wq_{h}")
                nc.sync.dma_start(
                    wq_h[:].rearrange("p (t c) -> p t c", c=128),
                    wq_d.ap()[:, h * 128 : (h + 1) * 128].rearrange(
                        "(t p) c -> p t c", p=128
                    ),
                )
                wq_tiles.append(wq_h)
            xbar(1)
            wo_sb = psb.tile([128, G * D], DT)  # [row-in-htile, htile*D]
            nc.sync.dma_start(
                wo_sb[:].rearrange("p (t c) -> p t c", c=D),
                wo_d.ap().rearrange("(t p) c -> p t c", p=128),
            )
            xbar(2)
            xbar(3)

            # ---- persistent SBUF tensors ---------------------------------
            latT = psb.tile([128, 2 * T], DT)   # [lat-in-tile, ltile*T]
            qT4 = psb.tile([128, G * T], DT)    # [hd, h*T + q]
            kT4 = psb.tile([128, G * T], DT)    # [hd, h*T + k]
            v_g = psb.tile([128, NSTRIP * G * (HD + 1)], DT)
            # softmax-denominator ones column per (strip, head)
            nc.vector.memset(
                v_g[:].rearrange("p (s g c) -> p s g c", g=G, c=HD + 1)[
                    :, :, :, HD : HD + 1
                ],
                1.0,
            )

            # ---- PE warm-up (p-state ramp) -------------------------------
            warm_sb = psb.tile([128, 128], F32)
            wps = pps.tile([128, 128], F32, tag="ps", name="warm_ps")
            for i in range(40):
                nc.tensor.matmul(
                    wps[:], ident[:], ident[:], start=(i == 0), stop=(i == 39)
                )
            nc.vector.tensor_copy(warm_sb[:], wps[:])

            # ---- filler unit builders ------------------------------------
            def u_lat(c, lt):
                def f():
                    ps = pps.tile([128, 512], F32, tag="ps", name=f"lat_{c}_{lt}")
                    c0 = c * 512
                    for dd in range(16):
                        nc.tensor.matmul(
                            ps[:],
                            wd_sb[:, dd * L + lt * 128 : dd * L + (lt + 1) * 128],
                            xkT[:, dd * T + c0 : dd * T + c0 + 512],
                            start=(dd == 0),
                            stop=(dd == 15),
                        )
                    nc.vector.tensor_copy(
                        latT[:, lt * T + c0 : lt * T + c0 + 512], ps[:]
                    )
                return f

            def u_qT(c, h):
                def f():
                    ps = pps.tile([128, 512], F32, tag="ps", name=f"q_{c}_{h}")
                    c0 = c * 512
                    for dd in range(16):
                        nc.tensor.matmul(
                            ps[:],
                            wq_tiles[h][:, dd * 128 : (dd + 1) * 128],
                            xkT[:, dd * T + c0 : dd * T + c0 + 512],
                            start=(dd == 0),
                            stop=(dd == 15),
                        )
                    nc.vector.tensor_copy(qT4[:, h * T + c0 : h * T + c0 + 512], ps[:])
                return f

            def u_kT(c, h):
                def f():
                    ps = pps.tile([128, 512], F32, tag="ps", name=f"k_{c}_{h}")
                    c0 = c * 512
                    for lt in range(2):
                        nc.tensor.matmul(
                            ps[:],
                            wuk_sb[:, lt * GW + h * 128 : lt * GW + (h + 1) * 128],
                            latT[:, lt * T + c0 : lt * T + c0 + 512],
                            start=(lt == 0),
                            stop=(lt == 1),
                        )
                    nc.vector.tensor_copy(kT4[:, h * T + c0 : h * T + c0 + 512], ps[:])
                return f

            def u_v(s):
                def f():
                    ps = pps.tile([128, 512], F32, tag="ps", name=f"v_{s}")
                    for lt in range(2):
                        nc.tensor.matmul(
                            ps[:],
                            latT[:, lt * T + s * 128 : lt * T + (s + 1) * 128],
                            wuv_sb[:, lt * GW : lt * GW + GW],
                            start=(lt == 0),
                            stop=(lt == 1),
                        )
                    base = s * G * (HD + 1)
                    nc.vector.tensor_copy(
                        v_g[:, base : base + G * (HD + 1)].rearrange(
                            "p (g c) -> p g c", c=HD + 1
                        )[:, :, :HD],
                        ps[:].rearrange("p (g c) -> p g c", c=HD),
                    )
                return f

            def u_oproj(ctxT_t, qb, qs, nb):
                def f():
                    ps = pps.tile([128, 512], F32, tag="ps", name=f"o_{qb}_{qs}_{nb}")
                    for h in range(G):
                        nc.tensor.matmul(
                            ps[:],
                            ctxT_t[:, (h * 4 + qs) * 128 : (h * 4 + qs + 1) * 128],
                            wo_sb[:, h * D + nb * 512 : h * D + (nb + 1) * 512],
                            start=(h == 0),
                            stop=(h == G - 1),
                        )
                    osb = psb.tile([128, 512], F32, tag="osb", bufs=4, name=f"osb_{qb}_{qs}_{nb}")
                    nc.vector.tensor_copy(osb[:], ps[:])
                    nc.sync.dma_start(
                        out_d.ap()[
                            qb * 512 + qs * 128 : qb * 512 + (qs + 1) * 128,
                            nb * 512 : (nb + 1) * 512,
                        ],
                        osb[:],
                    )
                return f

            def chunk_units(c):
                us = [u_lat(c, 0), u_lat(c, 1)]
                us += [u_qT(c, h) for h in range(G)]
                us += [u_kT(c, h) for h in range(G)]
                us += [u_v(s) for s in range(c * 4, c * 4 + 4)]
                return us

            def oproj_units(ctxT_t, qb):
                return [
                    u_oproj(ctxT_t, qb, qs, nb) for qs in range(4) for nb in range(4)
                ]

            def interleave(a, b):
                out, ia, ib = [], 0, 0
                while ia < len(a) or ib < len(b):
                    if ia < len(a):
                        out.append(a[ia]); ia += 1
                    if ib < len(b):
                        out.append(b[ib]); ib += 1
                return out

            # ---- attention for one query block, draining fillers ---------
            def attention(qb, fillers):
                nst = (qb + 1) * 4
                nslots = G * nst
                state = {"drained": 0, "slot": 0}

                def drain_to(target):
                    while state["drained"] < min(target, len(fillers)):
                        fillers[state["drained"]]()
                        state["drained"] += 1

                ctxT_t = psb.tile(
                    [128, G * 4 * 128], DT, tag="ctxT", bufs=2, name=f"ctxT_{qb}"
                )
                LA = 2
                for h in range(G):
                    e_tiles = {}

                    def escore(ks):
                        sps = pps.tile(
                            [128, 512], F32, tag="ps", name=f"s_{qb}_{h}_{ks}"
                        )
                        nc.tensor.matmul(
                            sps[:],
                            kT4[:, h * T + ks * 128 : h * T + (ks + 1) * 128],
                            qT4[:, h * T + qb * 512 : h * T + (qb + 1) * 512],
                            start=True,
                            stop=True,
                        )
                        e = psb.tile(
                            [128, 512], DT, tag="e", bufs=6, name=f"e_{qb}_{h}_{ks}"
                        )
                        nc.scalar.activation(
                            e[:], sps[:], mybir.ActivationFunctionType.Exp,
                            scale=float(SCALE),
                        )
                        if ks >= qb * 4:
                            jj = ks - qb * 4
                            nc.vector.tensor_mul(
                                e[:], e[:], mask_sb[:, jj * 512 : (jj + 1) * 512]
                            )
                        e_tiles[ks] = e

                    ctx = [
                        pctx.tile(
                            [128, HD + 1], F32, tag=f"c{qs}", bufs=1,
                            name=f"ctx_{qb}_{h}_{qs}",
                        )
                        for qs in range(4)
                    ]
                    for i in range(min(LA, nst)):
                        escore(i)
                    for ks in range(nst):
                        if ks + LA < nst:
                            escore(ks + LA)
                        e = e_tiles.pop(ks)
                        vb = ks * G * (HD + 1) + h * (HD + 1)
                        for qs in range(4):
                            nc.tensor.matmul(
                                ctx[qs][:],
                                e[:, qs * 128 : (qs + 1) * 128],
                                v_g[:, vb : vb + HD + 1],
                                start=(ks == 0),
                                stop=(ks == nst - 1),
                            )
                        state["slot"] += 1
                        drain_to(state["slot"] * len(fillers) // nslots)
                    # normalize + transpose into out-proj lhsT layout
                    for qs in range(4):
                        rec = psb.tile([128, 1], F32, tag="rec", bufs=2, name=f"rec_{qb}_{h}_{qs}")
                        nc.vector.reciprocal(rec[:], ctx[qs][:, HD : HD + 1])
                        ctxn = psb.tile([128, HD], DT, tag="ctxn", bufs=2, name=f"ctxn_{qb}_{h}_{qs}")
                        nc.vector.tensor_scalar_mul(ctxn[:], ctx[qs][:, :HD], rec[:])
                        tp = pps.tile([128, 128], DT, tag="tp", bufs=1, name=f"tp_{qb}_{h}_{qs}")
                        nc.tensor.transpose(tp[:], ctxn[:], ident[:])
                        nc.vector.tensor_copy(
                            ctxT_t[:, (h * 4 + qs) * 128 : (h * 4 + qs + 1) * 128],
                            tp[:],
                        )
                drain_to(len(fillers))
                return ctxT_t

            # ---- program ----------------------------------------------------
            for u in chunk_units(0):
                u()
            ctxT0 = attention(0, chunk_units(1))
            ctxT1 = attention(1, interleave(oproj_units(ctxT0, 0), chunk_units(2)))
            ctxT2 = attention(2, interleave(oproj_units(ctxT1, 1), chunk_units(3)))
            ctxT3 = attention(3, oproj_units(ctxT2, 2))
            for u in oproj_units(ctxT3, 3):
                u()

    nc.compile()
    return nc


_NC_CACHE = None


def _get_module():
    global _NC_CACHE
    if _NC_CACHE is None:
        _NC_CACHE = _build_module()
    return _NC_CACHE


def _host_prep(x, W_query, W_down, W_up_k, W_up_v, W_out):
    bf = lambda a: np.ascontiguousarray(a).astype(BF16)
    wd = bf(W_down)
    xb = [bf(x[0]), bf(x[1])]

    # causal triangle for the 4 diagonal strips of a 512-query block
    kk = np.arange(512).reshape(4, 128, 1)
    qq = np.arange(512).reshape(1, 1, 512)
    tri = (kk <= qq).astype(BF16)

    in_maps = []
    for j in range(N_CORES):
        b, g = divmod(j, 4)
        c0, c1 = g * GW, (g + 1) * GW
        in_maps.append(
            {
                "xk": xb[b],
                "wq": bf(W_query[:, c0:c1]),
                "wd": wd,
                "wuk": bf(W_up_k[:, c0:c1]),
                "wuv": bf(W_up_v[:, c0:c1]),
                "wo": bf(W_out[c0:c1, :]),
                "mask": tri,
            }
        )
    return in_maps


def kernel(x, W_query, W_down, W_up_k, W_up_v, W_out, _trace=False, _trace_kwargs=None):
    x = np.asarray(x, dtype=np.float32)
    in_maps = _host_prep(
        x,
        np.asarray(W_query, np.float32),
        np.asarray(W_down, np.float32),
        np.asarray(W_up_k, np.float32),
        np.asarray(W_up_v, np.float32),
        np.asarray(W_out, np.float32),
    )
    nc = _get_module()
    res = bass_utils.run_bass_kernel_spmd(
        nc, in_maps, core_ids=list(range(N_CORES)), trace=_trace,
        **(_trace_kwargs or {}),
    )
    y = np.zeros((B, T, D), np.float32)
    for j in range(N_CORES):
        b, _ = divmod(j, 4)
        y[b] += np.asarray(res.results[j]["out"], dtype=np.float32)
    kernel._last_results = res
    return y


# revision 3
# speedup vs baseline: 1.0482x; 1.0482x over previous
"""MultiHeadLatentAttention prefill kernel for 8 Trainium2 NeuronCores.

Sharding v2: head-group tensor parallelism. Core j handles batch j//4 and
head group j%4 (4 of 16 heads), computing attention for its heads over the
FULL sequence with causal strip-skipping (query block qb only visits key
strips <= diagonal, so scores/attn@v/exp do the minimal lower-triangle work).
Out-projection is row-parallel over heads: each core emits a full [T, D]
fp32 partial product; the host sums the 4 partials per batch at unshard time
(the all-reduce implied by row-parallel W_out, done on the host since the
SPMD harness has no cross-core collectives).

x^T is produced by the DMA xbar transpose engine (dma_start_transpose)
directly from DRAM, freeing the PE/DVE from 256 transpose+copy pairs.
All matmuls run bf16 with fp32 PSUM accumulation. Softmax skips
max-subtraction (|scores| <= ~1.3 here) and gets denominators from a ones
column appended to each head's V block. The emission schedule interleaves
projection / out-proj matmuls ("filler units") into the attention inner
loops so the PE never waits on the ACT engine's exp stream.
"""
import sys

sys.path.insert(0, "/opt/trn_rl_repo")

import numpy as np
import ml_dtypes

import concourse.bass as bass
import concourse.bacc as bacc
import concourse.mybir as mybir
import concourse.tile as tile
from concourse import bass_utils
from concourse.masks import make_identity

BF16 = ml_dtypes.bfloat16

B, T, D = 2, 2048, 2048
H, HD, L = 16, 128, 256
G = 4                      # heads per core
GW = G * HD                # 512: width of this core's head-group slice
NQB = 4                    # query blocks of 512
NSTRIP = 16                # key strips of 128
N_CORES = 8
SCALE = 1.0 / np.sqrt(HD)

DT = mybir.dt.bfloat16
F32 = mybir.dt.float32


def _build_module():
    nc = bacc.Bacc("TRN2", target_bir_lowering=False, debug=False)

    xk_d = nc.dram_tensor("xk", [T, D], DT, kind="ExternalInput")
    wq_d = nc.dram_tensor("wq", [D, GW], DT, kind="ExternalInput")
    wd_d = nc.dram_tensor("wd", [D, L], DT, kind="ExternalInput")
    wuk_d = nc.dram_tensor("wuk", [L, GW], DT, kind="ExternalInput")
    wuv_d = nc.dram_tensor("wuv", [L, GW], DT, kind="ExternalInput")
    wo_d = nc.dram_tensor("wo", [GW, D], DT, kind="ExternalInput")
    # triangular mask for the 4 diagonal strips of a 512-query block
    mask_d = nc.dram_tensor("mask", [4, 128, 512], DT, kind="ExternalInput")
    out_d = nc.dram_tensor("out", [T, D], F32, kind="ExternalOutput")

    with tile.TileContext(nc) as tc:
        with (
            tc.tile_pool(name="sb", bufs=1) as psb,
            tc.tile_pool(name="ps", bufs=4, space="PSUM") as pps,
            tc.tile_pool(name="ctx", bufs=1, space="PSUM") as pctx,
        ):
            ident = psb.tile([128, 128], DT)
            make_identity(nc, ident[:])

            # ---- DMA queue (single sync queue; xbar transposes and plain
            # loads must not run concurrently — they share the engines and
            # mixing modes costs badly). Order = consumer need order.
            xkT = psb.tile([128, 16 * T], DT)  # [d-in-tile, dtile*T]

            def xbar(c):
                # xkT[p, dd, q] = x[q, dd*128+p] for q in chunk c
                nc.sync.dma_start_transpose(
                    xkT[:].rearrange("p (t q) -> p t q", q=T)[
                        :, :, c * 512 : (c + 1) * 512
                    ],
                    xk_d.ap()[c * 512 : (c + 1) * 512, :],
                )

            xbar(0)
            wd_sb = psb.tile([128, 16 * L], DT)
            nc.sync.dma_start(
                wd_sb[:].rearrange("p (t c) -> p t c", c=L),
                wd_d.ap().rearrange("(t p) c -> p t c", p=128),
            )
            wq_tiles = []
            for h in range(G):
                wq_h = psb.tile([128, 16 * 128], DT, tag="wq", bufs=G, name=f"wq_{h}")
                nc.sync.dma_start(
                    wq_h[:].rearrange("p (t c) -> p t c", c=128),
                    wq_d.ap()[:, h * 128 : (h + 1) * 128].rearrange(
                        "(t p) c -> p t c", p=128
                    ),
                )
                wq_tiles.append(wq_h)
            wuk_sb = psb.tile([128, 2 * GW], DT)  # [lat-in-tile, ltile*GW]
            nc.sync.dma_start(
                wuk_sb[:].rearrange("p (t c) -> p t c", c=GW),
                wuk_d.ap().rearrange("(t p) c -> p t c", p=128),
            )
            wuv_sb = psb.tile([128, 2 * GW], DT)
            nc.sync.dma_start(
                wuv_sb[:].rearrange("p (t c) -> p t c", c=GW),
                wuv_d.ap().rearrange("(t p) c -> p t c", p=128),
            )
            mask_sb = psb.tile([128, 4 * 512], DT)
            for j in range(4):
                nc.sync.dma_start(
                    mask_sb[:, j * 512 : (j + 1) * 512], mask_d.ap()[j]
                )
            xbar(1)
            wo_sb = psb.tile([128, G * D], DT)  # [row-in-htile, htile*D]
            nc.sync.dma_start(
                wo_sb[:].rearrange("p (t c) -> p t c", c=D),
                wo_d.ap().rearrange("(t p) c -> p t c", p=128),
            )
            xbar(2)
            xbar(3)

            # ---- persistent SBUF tensors ---------------------------------
            latT = psb.tile([128, 2 * T], DT)   # [lat-in-tile, ltile*T]
            qT4 = psb.tile([128, G * T], DT)    # [hd, h*T + q]
            kT4 = psb.tile([128, G * T], DT)    # [hd, h*T + k]
            v_g = psb.tile([128, NSTRIP * G * (HD + 1)], DT)
            # softmax-denominator ones column per (strip, head)
            nc.vector.memset(
                v_g[:].rearrange("p (s g c) -> p s g c", g=G, c=HD + 1)[
                    :, :, :, HD : HD + 1
                ],
                1.0,
            )

            # ---- PE warm-up (p-state ramp) -------------------------------
            warm_sb = psb.tile([128, 128], F32)
            wps = pps.tile([128, 128], F32, tag="ps", name="warm_ps")
            for i in range(40):
                nc.tensor.matmul(
                    wps[:], ident[:], ident[:], start=(i == 0), stop=(i == 39)
                )
            nc.vector.tensor_copy(warm_sb[:], wps[:])

            # ---- filler unit builders ------------------------------------
            def u_lat(c, lt):
                def f():
                    ps = pps.tile([128, 512], F32, tag="ps", name=f"lat_{c}_{lt}")
                    c0 = c * 512
                    for dd in range(16):
                        nc.tensor.matmul(
                            ps[:],
                            wd_sb[:, dd * L + lt * 128 : dd * L + (lt + 1) * 128],
                            xkT[:, dd * T + c0 : dd * T + c0 + 512],
                            start=(dd == 0),
                            stop=(dd == 15),
                        )
                    nc.vector.tensor_copy(
                        latT[:, lt * T + c0 : lt * T + c0 + 512], ps[:]
                    )
                return f

            def u_qT(c, h):
                def f():
                    ps = pps.tile([128, 512], F32, tag="ps", name=f"q_{c}_{h}")
                    c0 = c * 512
                    for dd in range(16):
                        nc.tensor.matmul(
                            ps[:],
                            wq_tiles[h][:, dd * 128 : (dd + 1) * 128],
                            xkT[:, dd * T + c0 : dd * T + c0 + 512],
                            start=(dd == 0),
                            stop=(dd == 15),
                        )
                    nc.vector.tensor_copy(qT4[:, h * T + c0 : h * T + c0 + 512], ps[:])
                return f

            def u_kT(c, h):
                def f():
                    ps = pps.tile([128, 512], F32, tag="ps", name=f"k_{c}_{h}")
                    c0 = c * 512
                    for lt in range(2):
                        nc.tensor.matmul(
                            ps[:],
                            wuk_sb[:, lt * GW + h * 128 : lt * GW + (h + 1) * 128],
                            latT[:, lt * T + c0 : lt * T + c0 + 512],
                            start=(lt == 0),
                            stop=(lt == 1),
                        )
                    nc.vector.tensor_copy(kT4[:, h * T + c0 : h * T + c0 + 512], ps[:])
                return f

            def u_v(s):
                def f():
                    ps = pps.tile([128, 512], F32, tag="ps", name=f"v_{s}")
                    for lt in range(2):
                        nc.tensor.matmul(
                            ps[:],
                            latT[:, lt * T + s * 128 : lt * T + (s + 1) * 128],
                            wuv_sb[:, lt * GW : lt * GW + GW],
                            start=(lt == 0),
                            stop=(lt == 1),
                        )
                    base = s * G * (HD + 1)
                    nc.vector.tensor_copy(
                        v_g[:, base : base + G * (HD + 1)].rearrange(
                            "p (g c) -> p g c", c=HD + 1
                        )[:, :, :HD],
                        ps[:].rearrange("p (g c) -> p g c", c=HD),
                    )
                return f

            def u_oproj(ctxT_t, qb, qs, nb):
                def f():
                    ps = pps.tile([128, 512], F32, tag="ps", name=f"o_{qb}_{qs}_{nb}")
                    for h in range(G):
                        nc.tensor.matmul(
                            ps[:],
                            ctxT_t[:, (h * 4 + qs) * 128 : (h * 4 + qs + 1) * 128],
                            wo_sb[:, h * D + nb * 512 : h * D + (nb + 1) * 512],
                            start=(h == 0),
                            stop=(h == G - 1),
                        )
                    osb = psb.tile([128, 512], F32, tag="osb", bufs=4, name=f"osb_{qb}_{qs}_{nb}")
                    nc.vector.tensor_copy(osb[:], ps[:])
                    nc.sync.dma_start(
                        out_d.ap()[
                            qb * 512 + qs * 128 : qb * 512 + (qs + 1) * 128,
                            nb * 512 : (nb + 1) * 512,
                        ],
                        osb[:],
                    )
                return f

            def chunk_units(c):
                us = [u_lat(c, 0), u_lat(c, 1)]
                us += [u_qT(c, h) for h in range(G)]
                us += [u_kT(c, h) for h in range(G)]
                us += [u_v(s) for s in range(c * 4, c * 4 + 4)]
                return us

            def oproj_units(ctxT_t, qb):
                return [
                    u_oproj(ctxT_t, qb, qs, nb) for qs in range(4) for nb in range(4)
                ]

            def interleave(a, b):
                out, ia, ib = [], 0, 0
                while ia < len(a) or ib < len(b):
                    if ia < len(a):
                        out.append(a[ia]); ia += 1
                    if ib < len(b):
                        out.append(b[ib]); ib += 1
                return out

            # ---- attention for one query block, draining fillers ---------
            def attention(qb, fillers):
                nst = (qb + 1) * 4
                nslots = G * nst
                state = {"drained": 0, "slot": 0}

                def drain_to(target):
                    while state["drained"] < min(target, len(fillers)):
                        fillers[state["drained"]]()
                        state["drained"] += 1

                ctxT_t = psb.tile(
                    [128, G * 4 * 128], DT, tag="ctxT", bufs=2, name=f"ctxT_{qb}"
                )
                LA = 3
                for h in range(G):
                    e_tiles = {}

                    def escore(ks):
                        # diagonal strip jj>=1: queries < jj*128 are fully
                        # masked; trim scores/exp/mask to the live q range
                        jj = max(0, ks - qb * 4)
                        q0 = jj * 128
                        sps = pps.tile(
                            [128, 512], F32, tag="ps", name=f"s_{qb}_{h}_{ks}"
                        )
                        nc.tensor.matmul(
                            sps[:, q0:512],
                            kT4[:, h * T + ks * 128 : h * T + (ks + 1) * 128],
                            qT4[:, h * T + qb * 512 + q0 : h * T + (qb + 1) * 512],
                            start=True,
                            stop=True,
                        )
                        e = psb.tile(
                            [128, 512], DT, tag="e", bufs=6, name=f"e_{qb}_{h}_{ks}"
                        )
                        nc.scalar.activation(
                            e[:, q0:512], sps[:, q0:512],
                            mybir.ActivationFunctionType.Exp,
                            scale=float(SCALE),
                        )
                        if ks >= qb * 4:
                            nc.vector.tensor_mul(
                                e[:, q0:512], e[:, q0:512],
                                mask_sb[:, jj * 512 + q0 : (jj + 1) * 512],
                            )
                        e_tiles[ks] = e

                    ctx = [
                        pctx.tile(
                            [128, HD + 1], F32, tag=f"c{qs}", bufs=1,
                            name=f"ctx_{qb}_{h}_{qs}",
                        )
                        for qs in range(4)
                    ]
                    for i in range(min(LA, nst)):
                        escore(i)
                    for ks in range(nst):
                        if ks + LA < nst:
                            escore(ks + LA)
                        e = e_tiles.pop(ks)
                        vb = ks * G * (HD + 1) + h * (HD + 1)
                        for qs in range(4):
                            # strip ks contributes to q-substrip qs only when
                            # ks <= qb*4 + qs (causal); last contributor stops
                            if ks > qb * 4 + qs:
                                continue
                            nc.tensor.matmul(
                                ctx[qs][:],
                                e[:, qs * 128 : (qs + 1) * 128],
                                v_g[:, vb : vb + HD + 1],
                                start=(ks == 0),
                                stop=(ks == qb * 4 + qs),
                            )
                        state["slot"] += 1
                        drain_to(state["slot"] * len(fillers) // nslots)
                    # normalize + transpose into out-proj lhsT layout.
                    # DVE work first (batched), then PE transposes through the
                    # "ps" ring so consecutive transposes don't WAR-stall.
                    ctxns = []
                    for qs in range(4):
                        rec = psb.tile([128, 1], F32, tag="rec", bufs=2, name=f"rec_{qb}_{h}_{qs}")
                        nc.vector.reciprocal(rec[:], ctx[qs][:, HD : HD + 1])
                        ctxn = psb.tile([128, HD], DT, tag="ctxn", bufs=4, name=f"ctxn_{qb}_{h}_{qs}")
                        nc.vector.tensor_scalar_mul(ctxn[:], ctx[qs][:, :HD], rec[:])
                        ctxns.append(ctxn)
                    drain_to(state["drained"] + 1)
                    for qs in range(4):
                        # reuse the per-qs ctx bank (now dead until next h) as
                        # the transpose target: 4 distinct banks, no WAR chain
                        tp = pctx.tile([128, 128], DT, tag=f"c{qs}", bufs=1, name=f"tp_{qb}_{h}_{qs}")
                        nc.tensor.transpose(tp[:], ctxns[qs][:], ident[:])
                        nc.vector.tensor_copy(
                            ctxT_t[:, (h * 4 + qs) * 128 : (h * 4 + qs + 1) * 128],
                            tp[:],
                        )
                drain_to(len(fillers))
                return ctxT_t

            # ---- program ----------------------------------------------------
            for u in chunk_units(0):
                u()
            ctxT0 = attention(0, chunk_units(1))
            ctxT1 = attention(1, interleave(oproj_units(ctxT0, 0), chunk_units(2)))
            ctxT2 = attention(2, interleave(oproj_units(ctxT1, 1), chunk_units(3)))
            ctxT3 = attention(3, oproj_units(ctxT2, 2))
            for u in oproj_units(ctxT3, 3):
                u()

    nc.compile()
    return nc


_NC_CACHE = None


def _get_module():
    global _NC_CACHE
    if _NC_CACHE is None:
        _NC_CACHE = _build_module()
    return _NC_CACHE


def _host_prep(x, W_query, W_down, W_up_k, W_up_v, W_out):
    bf = lambda a: np.ascontiguousarray(a).astype(BF16)
    wd = bf(W_down)
    xb = [bf(x[0]), bf(x[1])]

    # causal triangle for the 4 diagonal strips of a 512-query block
    kk = np.arange(512).reshape(4, 128, 1)
    qq = np.arange(512).reshape(1, 1, 512)
    tri = (kk <= qq).astype(BF16)

    in_maps = []
    for j in range(N_CORES):
        b, g = divmod(j, 4)
        c0, c1 = g * GW, (g + 1) * GW
        in_maps.append(
            {
                "xk": xb[b],
                "wq": bf(W_query[:, c0:c1]),
                "wd": wd,
                "wuk": bf(W_up_k[:, c0:c1]),
                "wuv": bf(W_up_v[:, c0:c1]),
                "wo": bf(W_out[c0:c1, :]),
                "mask": tri,
            }
        )
    return in_maps


def kernel(x, W_query, W_down, W_up_k, W_up_v, W_out, _trace=False, _trace_kwargs=None):
    x = np.asarray(x, dtype=np.float32)
    in_maps = _host_prep(
        x,
        np.asarray(W_query, np.float32),
        np.asarray(W_down, np.float32),
        np.asarray(W_up_k, np.float32),
        np.asarray(W_up_v, np.float32),
        np.asarray(W_out, np.float32),
    )
    nc = _get_module()
    res = bass_utils.run_bass_kernel_spmd(
        nc, in_maps, core_ids=list(range(N_CORES)), trace=_trace,
        **(_trace_kwargs or {}),
    )
    y = np.zeros((B, T, D), np.float32)
    for j in range(N_CORES):
        b, _ = divmod(j, 4)
        y[b] += np.asarray(res.results[j]["out"], dtype=np.float32)
    kernel._last_results = res
    return y


# revision 4
# speedup vs baseline: 1.0482x; 1.0000x over previous
"""MultiHeadLatentAttention prefill kernel for 8 Trainium2 NeuronCores.

Sharding v2: head-group tensor parallelism. Core j handles batch j//4 and
head group j%4 (4 of 16 heads), computing attention for its heads over the
FULL sequence with causal strip-skipping (query block qb only visits key
strips <= diagonal, so scores/attn@v/exp do the minimal lower-triangle work).
Out-projection is row-parallel over heads: each core emits a full [T, D]
bf16 partial product; the host sums the 4 partials per batch at unshard time
(the all-reduce implied by row-parallel W_out, done on the host since the
SPMD harness has no cross-core collectives).

x^T: chunk 0 (needed immediately) is transposed on the PE from 4 plain strip
loads; chunks 1-3 come from the DMA xbar transpose engine
(dma_start_transpose ~19us/chunk of queue time) while attention runs.
All matmuls run bf16 with fp32 PSUM accumulation. Softmax skips
max-subtraction (|scores| <= ~1.3 here) and gets denominators from a ones
column appended to each head's V block. The emission schedule interleaves
projection / out-proj matmuls ("filler units") into the attention inner
loops so the PE never waits on the ACT engine's exp stream.
"""
import sys

sys.path.insert(0, "/opt/trn_rl_repo")

import numpy as np
import ml_dtypes

import concourse.bass as bass
import concourse.bacc as bacc
import concourse.mybir as mybir
import concourse.tile as tile
from concourse import bass_utils
from concourse.masks import make_identity

BF16 = ml_dtypes.bfloat16

B, T, D = 2, 2048, 2048
H, HD, L = 16, 128, 256
G = 4                      # heads per core
GW = G * HD                # 512: width of this core's head-group slice
NQB = 4                    # query blocks of 512
NSTRIP = 16                # key strips of 128
N_CORES = 8
SCALE = 1.0 / np.sqrt(HD)

DT = mybir.dt.bfloat16
F32 = mybir.dt.float32


def _build_module():
    nc = bacc.Bacc("TRN2", target_bir_lowering=False, debug=False)

    xk_d = nc.dram_tensor("xk", [T, D], DT, kind="ExternalInput")
    wq_d = nc.dram_tensor("wq", [D, GW], DT, kind="ExternalInput")
    wd_d = nc.dram_tensor("wd", [D, L], DT, kind="ExternalInput")
    wuk_d = nc.dram_tensor("wuk", [L, GW], DT, kind="ExternalInput")
    wuv_d = nc.dram_tensor("wuv", [L, GW], DT, kind="ExternalInput")
    wo_d = nc.dram_tensor("wo", [GW, D], DT, kind="ExternalInput")
    # triangular mask for the 4 diagonal strips of a 512-query block
    mask_d = nc.dram_tensor("mask", [4, 128, 512], DT, kind="ExternalInput")
    out_d = nc.dram_tensor("out", [T, D], DT, kind="ExternalOutput")

    with tile.TileContext(nc) as tc:
        with (
            tc.tile_pool(name="sb", bufs=1) as psb,
            tc.tile_pool(name="ps", bufs=4, space="PSUM") as pps,
            tc.tile_pool(name="ctx", bufs=1, space="PSUM") as pctx,
        ):
            ident = psb.tile([128, 128], DT)
            make_identity(nc, ident[:])

            # ---- DMA queue (single sync queue; xbar transposes and plain
            # loads must not run concurrently — they share the engines and
            # mixing modes costs badly). Order = consumer need order.
            xkT = psb.tile([128, 16 * T], DT)  # [d-in-tile, dtile*T]

            def xbar(c):
                # xkT[p, dd, q] = x[q, dd*128+p] for q in chunk c
                nc.sync.dma_start_transpose(
                    xkT[:].rearrange("p (t q) -> p t q", q=T)[
                        :, :, c * 512 : (c + 1) * 512
                    ],
                    xk_d.ap()[c * 512 : (c + 1) * 512, :],
                )

            # chunk 0 of x arrives as 4 plain strip loads (fast) and is
            # transposed on the PE; the xbar handles chunks 1-3 later, when
            # its ~19us/chunk queue occupancy overlaps attention.
            xs_tiles = []
            for s in range(4):
                xs = psb.tile([128, D], DT, tag="xs", bufs=4, name=f"xs_{s}")
                nc.sync.dma_start(xs[:], xk_d.ap()[s * 128 : (s + 1) * 128, :])
                xs_tiles.append(xs)
            wd_sb = psb.tile([128, 16 * L], DT)
            nc.sync.dma_start(
                wd_sb[:].rearrange("p (t c) -> p t c", c=L),
                wd_d.ap().rearrange("(t p) c -> p t c", p=128),
            )
            wuk_sb = psb.tile([128, 2 * GW], DT)  # [lat-in-tile, ltile*GW]
            nc.sync.dma_start(
                wuk_sb[:].rearrange("p (t c) -> p t c", c=GW),
                wuk_d.ap().rearrange("(t p) c -> p t c", p=128),
            )
            wuv_sb = psb.tile([128, 2 * GW], DT)
            nc.sync.dma_start(
                wuv_sb[:].rearrange("p (t c) -> p t c", c=GW),
                wuv_d.ap().rearrange("(t p) c -> p t c", p=128),
            )
            mask_sb = psb.tile([128, 4 * 512], DT)
            for j in range(4):
                nc.sync.dma_start(
                    mask_sb[:, j * 512 : (j + 1) * 512], mask_d.ap()[j]
                )
            wq_tiles = []
            for h in range(G):
                wq_h = psb.tile([128, 16 * 128], DT, tag="wq", bufs=G, name=f"wq_{h}")
                nc.sync.dma_start(
                    wq_h[:].rearrange("p (t c) -> p t c", c=128),
                    wq_d.ap()[:, h * 128 : (h + 1) * 128].rearrange(
                        "(t p) c -> p t c", p=128
                    ),
                )
                wq_tiles.append(wq_h)
            xbar(1)
            wo_sb = psb.tile([128, G * D], DT)  # [row-in-htile, htile*D]
            nc.sync.dma_start(
                wo_sb[:].rearrange("p (t c) -> p t c", c=D),
                wo_d.ap().rearrange("(t p) c -> p t c", p=128),
            )
            xbar(2)
            xbar(3)

            # ---- persistent SBUF tensors ---------------------------------
            latT = psb.tile([128, 2 * T], DT)   # [lat-in-tile, ltile*T]
            qT4 = psb.tile([128, G * T], DT)    # [hd, h*T + q]
            kT4 = psb.tile([128, G * T], DT)    # [hd, h*T + k]
            v_g = psb.tile([128, NSTRIP * G * (HD + 1)], DT)
            # softmax-denominator ones column per (strip, head)
            nc.vector.memset(
                v_g[:].rearrange("p (s g c) -> p s g c", g=G, c=HD + 1)[
                    :, :, :, HD : HD + 1
                ],
                1.0,
            )

            # ---- PE warm-up (p-state ramp) -------------------------------
            warm_sb = psb.tile([128, 128], F32)
            wps = pps.tile([128, 128], F32, tag="ps", name="warm_ps")
            for i in range(40):
                nc.tensor.matmul(
                    wps[:], ident[:], ident[:], start=(i == 0), stop=(i == 39)
                )
            nc.vector.tensor_copy(warm_sb[:], wps[:])

            # PE-transpose chunk 0 of x (strips 0-3), copies split DVE/ACT
            for s in range(4):
                for dd in range(16):
                    tp = pps.tile([128, 128], DT, tag="ps", name=f"xt_{s}_{dd}")
                    nc.tensor.transpose(
                        tp[:], xs_tiles[s][:, dd * 128 : (dd + 1) * 128], ident[:]
                    )
                    dst = xkT[:, dd * T + s * 128 : dd * T + (s + 1) * 128]
                    if dd % 2 == 0:
                        nc.vector.tensor_copy(dst, tp[:])
                    else:
                        nc.scalar.copy(dst, tp[:])

            # ---- filler unit builders ------------------------------------
            def _copy(eng, dst, srcap):
                if eng == 0:
                    nc.vector.tensor_copy(dst, srcap)
                else:
                    nc.scalar.copy(dst, srcap)

            def u_lat(c, lt, ce=0):
                def f():
                    ps = pps.tile([128, 512], F32, tag="ps", name=f"lat_{c}_{lt}")
                    c0 = c * 512
                    for dd in range(16):
                        nc.tensor.matmul(
                            ps[:],
                            wd_sb[:, dd * L + lt * 128 : dd * L + (lt + 1) * 128],
                            xkT[:, dd * T + c0 : dd * T + c0 + 512],
                            start=(dd == 0),
                            stop=(dd == 15),
                        )
                    _copy(ce, latT[:, lt * T + c0 : lt * T + c0 + 512], ps[:])
                return f

            def u_qT(c, h, ce=0):
                def f():
                    ps = pps.tile([128, 512], F32, tag="ps", name=f"q_{c}_{h}")
                    c0 = c * 512
                    for dd in range(16):
                        nc.tensor.matmul(
                            ps[:],
                            wq_tiles[h][:, dd * 128 : (dd + 1) * 128],
                            xkT[:, dd * T + c0 : dd * T + c0 + 512],
                            start=(dd == 0),
                            stop=(dd == 15),
                        )
                    _copy(ce, qT4[:, h * T + c0 : h * T + c0 + 512], ps[:])
                return f

            def u_kT(c, h, ce=0):
                def f():
                    ps = pps.tile([128, 512], F32, tag="ps", name=f"k_{c}_{h}")
                    c0 = c * 512
                    for lt in range(2):
                        nc.tensor.matmul(
                            ps[:],
                            wuk_sb[:, lt * GW + h * 128 : lt * GW + (h + 1) * 128],
                            latT[:, lt * T + c0 : lt * T + c0 + 512],
                            start=(lt == 0),
                            stop=(lt == 1),
                        )
                    _copy(ce, kT4[:, h * T + c0 : h * T + c0 + 512], ps[:])
                return f

            def u_v(s):
                def f():
                    ps = pps.tile([128, 512], F32, tag="ps", name=f"v_{s}")
                    for lt in range(2):
                        nc.tensor.matmul(
                            ps[:],
                            latT[:, lt * T + s * 128 : lt * T + (s + 1) * 128],
                            wuv_sb[:, lt * GW : lt * GW + GW],
                            start=(lt == 0),
                            stop=(lt == 1),
                        )
                    base = s * G * (HD + 1)
                    nc.vector.tensor_copy(
                        v_g[:, base : base + G * (HD + 1)].rearrange(
                            "p (g c) -> p g c", c=HD + 1
                        )[:, :, :HD],
                        ps[:].rearrange("p (g c) -> p g c", c=HD),
                    )
                return f

            def u_oproj(ctxT_t, qb, qs, nb):
                def f():
                    ps = pps.tile([128, 512], F32, tag="ps", name=f"o_{qb}_{qs}_{nb}")
                    for h in range(G):
                        nc.tensor.matmul(
                            ps[:],
                            ctxT_t[:, (h * 4 + qs) * 128 : (h * 4 + qs + 1) * 128],
                            wo_sb[:, h * D + nb * 512 : h * D + (nb + 1) * 512],
                            start=(h == 0),
                            stop=(h == G - 1),
                        )
                    osb = psb.tile([128, 512], DT, tag="osb", bufs=4, name=f"osb_{qb}_{qs}_{nb}")
                    nc.vector.tensor_copy(osb[:], ps[:])
                    nc.sync.dma_start(
                        out_d.ap()[
                            qb * 512 + qs * 128 : qb * 512 + (qs + 1) * 128,
                            nb * 512 : (nb + 1) * 512,
                        ],
                        osb[:],
                    )
                return f

            def chunk_units(c):
                # during qb0/qb1 the ACT engine is mostly idle: route every
                # other PSUM->SBUF copy through it (chunk 3 runs while exp
                # load is high, keep it on DVE)
                alt = c in (1, 2)
                us = [u_lat(c, 0, ce=0), u_lat(c, 1, ce=1 if alt else 0)]
                us += [u_qT(c, h, ce=(h % 2 if alt else 0)) for h in range(G)]
                us += [u_kT(c, h, ce=(h % 2 if alt else 0)) for h in range(G)]
                us += [u_v(s) for s in range(c * 4, c * 4 + 4)]
                return us

            def oproj_units(ctxT_t, qb):
                return [
                    u_oproj(ctxT_t, qb, qs, nb) for qs in range(4) for nb in range(4)
                ]

            def interleave(a, b):
                out, ia, ib = [], 0, 0
                while ia < len(a) or ib < len(b):
                    if ia < len(a):
                        out.append(a[ia]); ia += 1
                    if ib < len(b):
                        out.append(b[ib]); ib += 1
                return out

            # ---- attention for one query block, draining fillers ---------
            def attention(qb, fillers):
                nst = (qb + 1) * 4
                nslots = G * nst
                state = {"drained": 0, "slot": 0}

                def drain_to(target):
                    while state["drained"] < min(target, len(fillers)):
                        fillers[state["drained"]]()
                        state["drained"] += 1

                ctxT_t = psb.tile(
                    [128, G * 4 * 128], DT, tag="ctxT", bufs=2, name=f"ctxT_{qb}"
                )
                LA = 3
                for h in range(G):
                    e_tiles = {}

                    def escore(ks):
                        # diagonal strip jj>=1: queries < jj*128 are fully
                        # masked; trim scores/exp/mask to the live q range
                        jj = max(0, ks - qb * 4)
                        q0 = jj * 128
                        sps = pps.tile(
                            [128, 512], F32, tag="ps", name=f"s_{qb}_{h}_{ks}"
                        )
                        nc.tensor.matmul(
                            sps[:, q0:512],
                            kT4[:, h * T + ks * 128 : h * T + (ks + 1) * 128],
                            qT4[:, h * T + qb * 512 + q0 : h * T + (qb + 1) * 512],
                            start=True,
                            stop=True,
                        )
                        e = psb.tile(
                            [128, 512], DT, tag="e", bufs=6, name=f"e_{qb}_{h}_{ks}"
                        )
                        nc.scalar.activation(
                            e[:, q0:512], sps[:, q0:512],
                            mybir.ActivationFunctionType.Exp,
                            scale=float(SCALE),
                        )
                        if ks >= qb * 4:
                            nc.vector.tensor_mul(
                                e[:, q0:512], e[:, q0:512],
                                mask_sb[:, jj * 512 + q0 : (jj + 1) * 512],
                            )
                        e_tiles[ks] = e

                    ctx = [
                        pctx.tile(
                            [128, HD + 1], F32, tag=f"c{qs}", bufs=1,
                            name=f"ctx_{qb}_{h}_{qs}",
                        )
                        for qs in range(4)
                    ]
                    for i in range(min(LA, nst)):
                        escore(i)
                    for ks in range(nst):
                        if ks + LA < nst:
                            escore(ks + LA)
                        e = e_tiles.pop(ks)
                        vb = ks * G * (HD + 1) + h * (HD + 1)
                        for qs in range(4):
                            # strip ks contributes to q-substrip qs only when
                            # ks <= qb*4 + qs (causal); last contributor stops
                            if ks > qb * 4 + qs:
                                continue
                            nc.tensor.matmul(
                                ctx[qs][:],
                                e[:, qs * 128 : (qs + 1) * 128],
                                v_g[:, vb : vb + HD + 1],
                                start=(ks == 0),
                                stop=(ks == qb * 4 + qs),
                            )
                        state["slot"] += 1
                        drain_to(state["slot"] * len(fillers) // nslots)
                    # normalize + transpose into out-proj lhsT layout.
                    # DVE work first (batched), then PE transposes through the
                    # "ps" ring so consecutive transposes don't WAR-stall.
                    ctxns = []
                    for qs in range(4):
                        rec = psb.tile([128, 1], F32, tag="rec", bufs=2, name=f"rec_{qb}_{h}_{qs}")
                        nc.vector.reciprocal(rec[:], ctx[qs][:, HD : HD + 1])
                        ctxn = psb.tile([128, HD], DT, tag="ctxn", bufs=4, name=f"ctxn_{qb}_{h}_{qs}")
                        nc.vector.tensor_scalar_mul(ctxn[:], ctx[qs][:, :HD], rec[:])
                        ctxns.append(ctxn)
                    drain_to(state["drained"] + 1)
                    for qs in range(4):
                        # reuse the per-qs ctx bank (now dead until next h) as
                        # the transpose target: 4 distinct banks, no WAR chain
                        tp = pctx.tile([128, 128], DT, tag=f"c{qs}", bufs=1, name=f"tp_{qb}_{h}_{qs}")
                        nc.tensor.transpose(tp[:], ctxns[qs][:], ident[:])
                        nc.vector.tensor_copy(
                            ctxT_t[:, (h * 4 + qs) * 128 : (h * 4 + qs + 1) * 128],
                            tp[:],
                        )
                drain_to(len(fillers))
                return ctxT_t

            # ---- program ----------------------------------------------------
            c0_units = [u_lat(0, 0), u_lat(0, 1)]
            c0_units += [u_kT(0, h) for h in range(G)]
            c0_units += [u_v(s) for s in range(4)]
            c0_units += [u_qT(0, h) for h in range(G)]
            for u in c0_units:
                u()
            ctxT0 = attention(0, chunk_units(1))
            ctxT1 = attention(1, interleave(oproj_units(ctxT0, 0), chunk_units(2)))
            ctxT2 = attention(2, interleave(oproj_units(ctxT1, 1), chunk_units(3)))
            ctxT3 = attention(3, oproj_units(ctxT2, 2))
            for u in oproj_units(ctxT3, 3):
                u()

    nc.compile()
    return nc


_NC_CACHE = None


def _get_module():
    global _NC_CACHE
    if _NC_CACHE is None:
        _NC_CACHE = _build_module()
    return _NC_CACHE


def _host_prep(x, W_query, W_down, W_up_k, W_up_v, W_out):
    bf = lambda a: np.ascontiguousarray(a).astype(BF16)
    wd = bf(W_down)
    xb = [bf(x[0]), bf(x[1])]

    # causal triangle for the 4 diagonal strips of a 512-query block
    kk = np.arange(512).reshape(4, 128, 1)
    qq = np.arange(512).reshape(1, 1, 512)
    tri = (kk <= qq).astype(BF16)

    in_maps = []
    for j in range(N_CORES):
        b, g = divmod(j, 4)
        c0, c1 = g * GW, (g + 1) * GW
        in_maps.append(
            {
                "xk": xb[b],
                "wq": bf(W_query[:, c0:c1]),
                "wd": wd,
                "wuk": bf(W_up_k[:, c0:c1]),
                "wuv": bf(W_up_v[:, c0:c1]),
                "wo": bf(W_out[c0:c1, :]),
                "mask": tri,
            }
        )
    return in_maps


def kernel(x, W_query, W_down, W_up_k, W_up_v, W_out, _trace=False, _trace_kwargs=None):
    x = np.asarray(x, dtype=np.float32)
    in_maps = _host_prep(
        x,
        np.asarray(W_query, np.float32),
        np.asarray(W_down, np.float32),
        np.asarray(W_up_k, np.float32),
        np.asarray(W_up_v, np.float32),
        np.asarray(W_out, np.float32),
    )
    nc = _get_module()
    res = bass_utils.run_bass_kernel_spmd(
        nc, in_maps, core_ids=list(range(N_CORES)), trace=_trace,
        **(_trace_kwargs or {}),
    )
    y = np.zeros((B, T, D), np.float32)
    for j in range(N_CORES):
        b, _ = divmod(j, 4)
        y[b] += np.asarray(res.results[j]["out"], dtype=np.float32)
    kernel._last_results = res
    return y
